# revision 1
# baseline (speedup 1.0000x reference)
"""GRU cell kernel for Trainium2, data-parallel across 8 NeuronCores.

Reference computation (per batch row):
    concat = [h_prev, x]                       # [B, 2048]
    z = sigmoid(concat @ W_z.T + b_z)          # [B, 1024]
    r = sigmoid(concat @ W_r.T + b_r)
    h_tilde = tanh([r*h_prev, x] @ W_h.T + b_h)
    h = (1-z)*h_prev + z*h_tilde

Sharding: batch dim (8192) split 1024/core; weights replicated.
Layout on device is feature-major ([feature, batch]) so the batch is the
matmul moving dimension (N=512 per PSUM bank) and the hidden units are the
PSUM partition dim. Host transposes in/out.

mm_dtype selects the matmul precision:
  f32r — TF32 PE mode, fp32 storage (rel err ~1e-4)
  bf16 — bf16 operands (weights/acts), fp32 h_prev kept for elementwise
  f32  — true fp32 matmuls (4x slower on PE)
"""

import numpy as np

import concourse.bacc as bacc
import concourse.bass as bass
import concourse.mybir as mybir
import concourse.tile as tile
from concourse import bass_utils

P = 128
B = 8192
I = 1024
H = 1024
K = I + H            # 2048 contraction
NCORES = 8
BS = B // NCORES     # 1024 batch rows per core
MT = H // P          # 8 m-tiles (hidden units)
KT = K // P          # 16 k-chunks
NFREE = 512          # matmul moving free dim (one PSUM bank of fp32)
NT = BS // NFREE     # 2 n-tiles per core

F32 = mybir.dt.float32
F32R = mybir.dt.float32r
BF16 = mybir.dt.bfloat16

AF = mybir.ActivationFunctionType


def build_kernel(mm_dtype: str = "f32r"):
    """Build the per-core Bass kernel. Returns compiled nc."""
    mdt = {"f32r": F32R, "f32": F32, "bf16": BF16}[mm_dtype]
    bf16 = mm_dtype == "bf16"
    nc = bacc.Bacc("TRN2", target_bir_lowering=False, debug=False)

    # DRAM I/O (per-core shapes). Matmul-feeding tensors carry the matmul
    # dtype (f32r is the same bits as f32 on the host side).
    xT = nc.dram_tensor("xT", [I, BS], mdt, kind="ExternalInput").ap()
    hT = nc.dram_tensor("hT", [H, BS], mdt, kind="ExternalInput").ap()
    if bf16:  # separate fp32 copy of h_prev for the elementwise path
        hTf = nc.dram_tensor("hTf", [H, BS], F32, kind="ExternalInput").ap()
    Wz = nc.dram_tensor("Wz", [MT, P, K], mdt, kind="ExternalInput").ap()
    Wr = nc.dram_tensor("Wr", [MT, P, K], mdt, kind="ExternalInput").ap()
    Wh = nc.dram_tensor("Wh", [MT, P, K], mdt, kind="ExternalInput").ap()
    bz = nc.dram_tensor("bz", [P, MT], F32, kind="ExternalInput").ap()
    br = nc.dram_tensor("br", [P, MT], F32, kind="ExternalInput").ap()
    bh = nc.dram_tensor("bh", [P, MT], F32, kind="ExternalInput").ap()
    out = nc.dram_tensor("out", [H, BS], F32, kind="ExternalOutput").ap()

    def ew(ap):
        """fp32 view of an f32r AP for elementwise use (same bits)."""
        return ap.bitcast(F32) if mdt == F32R else ap

    with tile.TileContext(nc) as tc:
        with (
            tc.tile_pool(name="acts", bufs=1) as acts,
            tc.tile_pool(name="gates", bufs=1) as gates,
            tc.tile_pool(name="wpool", bufs=5) as wpool,
            tc.tile_pool(name="opool", bufs=6) as opool,
            tc.tile_pool(name="ppool", bufs=8, space="PSUM") as ppool,
        ):
            # Biases first — they gate the first sigmoid (psum recycling).
            # Scalar HWDGE ring so they don't sit behind the act loads.
            bz_sb = acts.tile([P, MT], F32)
            br_sb = acts.tile([P, MT], F32)
            bh_sb = acts.tile([P, MT], F32)
            nc.scalar.dma_start(br_sb[:], br)
            nc.scalar.dma_start(bz_sb[:], bz)
            nc.scalar.dma_start(bh_sb[:], bh)

            # First two weight tiles go at the HEAD of the sync HWDGE ring:
            # within a ring DMAs drain FIFO, so they get full SDMA bandwidth
            # before the act loads start, instead of a round-robin share.
            # (The SWDGE queue used for the remaining tiles takes ~6us to
            # spin up anyway.)
            w_pre = {("r", i): wpool.tile([P, K], mdt, tag="w", name=f"wf{i}")
                     for i in range(6)}
            nc.sync.dma_start(w_pre[("r", 0)][:], Wr[0])
            nc.scalar.dma_start(w_pre[("r", 1)][:], Wr[1])
            nc.scalar.dma_start(w_pre[("r", 3)][:], Wr[3])

            # Pre-warm the ACT sigmoid/tanh table set during the DMA fill so
            # the first real sigmoid doesn't pay the ~2.7us ACT_TABLE_LOAD.
            # Reads its own uninitialized tile — no DMA dependency, result
            # discarded — so it cannot stall the scalar ring's weight DMAs.
            warm = acts.tile([P, 1], F32)
            nc.scalar.activation(warm[:], warm[:], AF.Sigmoid)

            # Persistent activations, feature-major: [p, ko, batch]
            xT_sb = acts.tile([P, I // P, BS], mdt)
            hT_sb = acts.tile([P, H // P, BS], mdt)
            hTf_sb = (acts.tile([P, H // P, BS], F32, name="hTf_sb")
                      if bf16 else None)
            # Load per (tensor, batch-half), n=0 halves first, so the first
            # PSUM groups (n=0) are gated on half the act bytes. One DMA per
            # half: each dma_start costs ~600ns of descriptor-gen serialized
            # on the sync sequencer, so many small chunk loads would delay
            # the bytes the first PSUM group needs. Weight DMAs ride the
            # idle GpSimd SWDGE queue so they don't serialize with act
            # loads or compute issue.
            xT_r = xT.rearrange("(ko p) b -> p ko b", p=P)
            hT_r = hT.rearrange("(ko p) b -> p ko b", p=P)
            hTf_r = hTf.rearrange("(ko p) b -> p ko b", p=P) if bf16 else None
            # Interleave the next R-gate weight tiles into the sync FIFO so
            # they drain right AFTER the bytes the first PSUM groups need,
            # instead of contending with them from the gpsimd ring.
            for n in range(NT):
                ns = slice(n * NFREE, (n + 1) * NFREE)
                nc.sync.dma_start(hT_sb[:, :, ns], hT_r[:, :, ns])
                if n == 0:
                    # w2 drains before xT-n0: the interleaved ramp consumes
                    # (w0..w3, hT-n0) first and must not head-of-line block.
                    nc.sync.dma_start(w_pre[("r", 2)][:], Wr[2])
                nc.sync.dma_start(xT_sb[:, :, ns], xT_r[:, :, ns])
                if n == 1:
                    nc.sync.dma_start(w_pre[("r", 4)][:], Wr[4])
            if bf16:
                for half in range(2):
                    ks = slice(half * 4, (half + 1) * 4)
                    nc.sync.dma_start(hTf_sb[:, ks, :], hTf_r[:, ks, :])
            nc.sync.dma_start(w_pre[("r", 5)][:], Wr[5])

            # Gate results, feature-major
            z_sb = gates.tile([P, MT, BS], F32)
            rh_sb = gates.tile([P, MT, BS], mdt)

            def hprev_ew(mt, ns):
                """fp32-precision h_prev slice for elementwise use."""
                if bf16:
                    return hTf_sb[:, mt, ns]
                return ew(hT_sb[:, mt, ns])

            def rhs_chunk(k, base, width, stage):
                """Moving operand [128, width] for contraction chunk k."""
                if k < H // P:
                    src = rh_sb if stage == "h" else hT_sb
                    return src[:, k, base:base + width]
                return xT_sb[:, k - H // P, base:base + width]

            def gate(stage, Wd, b_sb):
                if stage == "r":
                    # Interleaved ramp: open 4 PSUM groups (mt0-3, n=0),
                    # k-outer across them, so the PE runs 32 matmuls on the
                    # already-arrived h_prev half while the x half of the
                    # batch (and later weights) are still in flight.
                    NG = 4
                    ws = [w_pre[("r", g)] for g in range(NG)]
                    pss = [ppool.tile([P, NFREE], F32, tag="ps",
                                      name=f"psri{g}") for g in range(NG)]
                    for k in range(KT):
                        for g in range(NG):
                            nc.tensor.matmul(
                                pss[g], ws[g][:, k * P:(k + 1) * P],
                                rhs_chunk(k, 0, NFREE, stage),
                                start=(k == 0), stop=(k == KT - 1))
                    for g in range(NG):
                        ns0 = slice(0, NFREE)
                        r_tmp = opool.tile([P, NFREE], F32, tag="rt",
                                           name=f"rti{g}")
                        nc.scalar.activation(
                            r_tmp, pss[g], AF.Sigmoid, bias=b_sb[:, g:g + 1])
                        nc.vector.tensor_mul(
                            rh_sb[:, g, ns0], r_tmp, hprev_ew(g, ns0))
                    plan = ([(mt, 1) for mt in range(NG)]
                            + [(mt, n) for mt in range(NG, MT)
                               for n in range(NT)])
                else:
                    plan = [(mt, n) for mt in range(MT) for n in range(NT)]
                for mt, n in plan:
                    w_sb = w_pre.get((stage, mt))
                    if w_sb is None:
                        w_sb = wpool.tile([P, K], mdt, tag="w")
                        nc.gpsimd.dma_start(w_sb[:], Wd[mt])
                        w_pre[(stage, mt)] = w_sb
                    if True:
                        # Split the very last group so its activation+combine
                        # +store chain pipelines instead of sitting fully
                        # exposed after the final matmul.
                        last = stage == "h" and mt == MT - 1 and n == NT - 1
                        nsub = 2 if last else 1
                        width = NFREE // nsub
                        for s in range(nsub):
                            base = n * NFREE + s * width
                            ps = ppool.tile([P, width], F32, tag="ps",
                                            name=f"ps{mt}_{n}_{s}")
                            for k in range(KT):
                                nc.tensor.matmul(
                                    ps,
                                    w_sb[:, k * P:(k + 1) * P],
                                    rhs_chunk(k, base, width, stage),
                                    start=(k == 0),
                                    stop=(k == KT - 1),
                                )
                            ns = slice(base, base + width)
                            bias = b_sb[:, mt:mt + 1]
                            if stage == "r":
                                # r -> rh = r * h_prev, in matmul dtype
                                r_tmp = opool.tile([P, width], F32, tag="rt")
                                nc.scalar.activation(
                                    r_tmp, ps, AF.Sigmoid, bias=bias)
                                nc.vector.tensor_mul(
                                    rh_sb[:, mt, ns], r_tmp, hprev_ew(mt, ns))
                            elif stage == "z":
                                nc.scalar.activation(
                                    z_sb[:, mt, ns], ps, AF.Sigmoid, bias=bias)
                            else:  # h = h_prev + z*(tanh(pre) - h_prev)
                                ht = opool.tile([P, width], F32, tag="ht",
                                                name=f"ht{mt}_{n}_{s}")
                                nc.scalar.activation(
                                    ht, ps, AF.Tanh, bias=bias)
                                nc.vector.tensor_sub(ht, ht, hprev_ew(mt, ns))
                                nc.vector.tensor_mul(ht, ht, z_sb[:, mt, ns])
                                nc.vector.tensor_add(ht, ht, hprev_ew(mt, ns))
                                nc.sync.dma_start(
                                    out[mt * P:(mt + 1) * P, ns], ht)

            gate("r", Wr, br_sb)
            gate("z", Wz, bz_sb)
            gate("h", Wh, bh_sb)

    nc.compile()
    return nc


def _prep_inputs(x, h_prev, W_z, b_z, W_r, b_r, W_h, b_h, np_dtype=np.float32):
    """Host-side relayout: feature-major activations, m-tiled weights."""
    bf16 = np_dtype != np.float32

    def prep_w(W):
        # want w[mt, p, ko*128+m] = W[mt*128+m, ko*128+p]
        W4 = W.reshape(MT, P, KT, P)          # [mt, m, ko, p]
        return np.ascontiguousarray(
            W4.transpose(0, 3, 2, 1)).reshape(MT, P, K).astype(np_dtype)

    def prep_b(b):
        return np.ascontiguousarray(b.reshape(MT, P).T)

    xT = np.ascontiguousarray(x.T).astype(np_dtype)       # [I, B]
    hTf = np.ascontiguousarray(h_prev.T)                  # [H, B] f32
    hT = hTf.astype(np_dtype)
    shared = {
        "Wz": prep_w(W_z), "Wr": prep_w(W_r), "Wh": prep_w(W_h),
        "bz": prep_b(b_z), "br": prep_b(b_r), "bh": prep_b(b_h),
    }
    in_maps = []
    for c in range(NCORES):
        bs = slice(c * BS, (c + 1) * BS)
        m = dict(shared)
        m["xT"] = np.ascontiguousarray(xT[:, bs])
        m["hT"] = np.ascontiguousarray(hT[:, bs])
        if bf16:
            m["hTf"] = np.ascontiguousarray(hTf[:, bs])
        in_maps.append(m)
    return in_maps


def run(inputs, mm_dtype="bf16", trace=False, **run_kwargs):
    """Compile + run on 8 cores. Returns (output [B,H] f32, BassKernelResults)."""
    import ml_dtypes
    nc = build_kernel(mm_dtype)
    np_dtype = ml_dtypes.bfloat16 if mm_dtype == "bf16" else np.float32
    in_maps = _prep_inputs(**inputs, np_dtype=np_dtype)
    res = bass_utils.run_bass_kernel_spmd(
        nc, in_maps, core_ids=list(range(NCORES)), trace=trace, **run_kwargs)
    outT = np.concatenate(
        [res.results[c]["out"] for c in range(NCORES)], axis=1)  # [H, B]
    return np.ascontiguousarray(outT.T), res


def kernel(**inputs) -> np.ndarray:
    import time as _time
    try:
        out, _ = run(inputs)
    except Exception:
        # The axon-tunneled device occasionally reports a transient
        # "unrecoverable" state right after a crashed session; a fresh
        # attempt after a short pause recovers.
        _time.sleep(15)
        out, _ = run(inputs)
    return out



# revision 4
# speedup vs baseline: 1.5828x; 1.5828x over previous
"""GRU cell kernel for Trainium2, data-parallel across 8 NeuronCores.

Reference computation (per batch row):
    concat = [h_prev, x]                       # [B, 2048]
    z = sigmoid(concat @ W_z.T + b_z)          # [B, 1024]
    r = sigmoid(concat @ W_r.T + b_r)
    h_tilde = tanh([r*h_prev, x] @ W_h.T + b_h)
    h = (1-z)*h_prev + z*h_tilde

Sharding: batch dim (8192) split 1024/core; weights replicated.

Matmuls run in fp8 e4m3 with MatmulPerfMode.DoubleRow: each instruction
consumes TWO 128-deep contraction chunks ([128, 2, free] operands), at
0.5 PE cycles per moving column -> 4x bf16 MAC throughput. Weights are
prescaled by 256 on the host so they sit in e4m3's normal range
(|W| ~ 0.022 would otherwise be denormal); the 1/256 descale folds into
the ACT activation's `scale` operand.  Elementwise runs in bf16 (2x DVE
rate), h_prev is kept as a bf16 copy for the state-carry path, and
c = (1-z)*h_prev is precomputed during the z-stage so the h-stage tail
only needs two DVE ops per tile.  Expected rel err ~1.7e-2 (fp8 operand
quantization, dominated by the h/z gates); `fp8wh`/`fp8whz` variants add
a second weight-residual matmul pass on W_h (and W_z) to cut it to
~1.5e-2/~1.3e-2 at +13.7us PE each.

All DRAM layouts are partition-major so every DMA is contiguous per
partition (>=2KB descriptors): the baseline trace showed the 17us ramp
was descriptor/latency-bound, not bandwidth-bound.
"""

import numpy as np

import concourse.bacc as bacc
import concourse.bass as bass
import concourse.mybir as mybir
import concourse.tile as tile
from concourse import bass_utils

P = 128
B = 8192
I = 1024
H = 1024
K = I + H            # 2048 contraction
NCORES = 8
BS = B // NCORES     # 1024 batch rows per core
MT = H // P          # 8 m-tiles (hidden units)
KT = K // P          # 16 k-chunks
KP = KT // 2         # 8 DoubleRow pairs (0-3 h-part, 4-7 x-part)
NF = 512             # matmul moving free dim (one PSUM bank of fp32)
NT = BS // NF        # 2 n-tiles per core
WSCALE = 256.0       # host-side weight prescale for e4m3 range

F32 = mybir.dt.float32
BF16 = mybir.dt.bfloat16
F8 = mybir.dt.float8e4

AF = mybir.ActivationFunctionType
DR = mybir.MatmulPerfMode.DoubleRow


def build_kernel(variant: str = "fp8"):
    """Build the per-core Bass kernel. Returns compiled nc."""
    wlo_h = variant in ("fp8wh", "fp8whz")
    wlo_z = variant == "fp8whz"
    nc = bacc.Bacc("TRN2", target_bir_lowering=False, debug=False)

    # DRAM I/O (per-core shapes), all partition-major.
    x8d = nc.dram_tensor("x8", [P, NT, KP, NF], F8, kind="ExternalInput").ap()
    h8d = nc.dram_tensor("h8", [P, NT, KP, NF], F8, kind="ExternalInput").ap()
    hpbd = nc.dram_tensor("hpb", [P, NT, MT, NF], BF16,
                          kind="ExternalInput").ap()
    Wrd = nc.dram_tensor("Wr", [P, MT, KT, P], F8, kind="ExternalInput").ap()
    Wzd = nc.dram_tensor("Wz", [P, MT, KT, P], F8, kind="ExternalInput").ap()
    Whd = nc.dram_tensor("Wh", [P, MT, KT, P], F8, kind="ExternalInput").ap()
    Whl = (nc.dram_tensor("Whl", [P, MT, KT, P], F8, kind="ExternalInput").ap()
           if wlo_h else None)
    Wzl = (nc.dram_tensor("Wzl", [P, MT, KT, P], F8, kind="ExternalInput").ap()
           if wlo_z else None)
    brd = nc.dram_tensor("br", [P, MT], F32, kind="ExternalInput").ap()
    bzd = nc.dram_tensor("bz", [P, MT], F32, kind="ExternalInput").ap()
    bhd = nc.dram_tensor("bh", [P, MT], F32, kind="ExternalInput").ap()
    out = nc.dram_tensor("out", [H, BS], BF16, kind="ExternalOutput").ap()

    with tile.TileContext(nc) as tc:
        with (
            tc.tile_pool(name="acts", bufs=1) as acts,
            tc.tile_pool(name="gates", bufs=1) as gates,
            tc.tile_pool(name="opool", bufs=8) as opool,
            tc.tile_pool(name="ppool", bufs=8, space="PSUM") as ppool,
        ):
            # Persistent SBUF state
            x8_sb = acts.tile([P, NT, KP, NF], F8)
            h8_sb = acts.tile([P, NT, KP, NF], F8)
            hpb_sb = acts.tile([P, NT, MT, NF], BF16)
            wr_sb = acts.tile([P, MT, KT, P], F8)
            wz_sb = acts.tile([P, MT, KT, P], F8)
            wh_sb = acts.tile([P, MT, KT, P], F8)
            whl_sb = acts.tile([P, MT, KT, P], F8, name="whl") if wlo_h else None
            wzl_sb = acts.tile([P, MT, KT, P], F8, name="wzl") if wlo_z else None
            br_sb = acts.tile([P, MT], F32)
            bz_sb = acts.tile([P, MT], F32)
            bh_sb = acts.tile([P, MT], F32)
            r_sb = gates.tile([P, NT, MT, NF], BF16)
            z_sb = gates.tile([P, NT, MT, NF], BF16)
            c_sb = gates.tile([P, NT, MT, NF], BF16)
            rh_sb = gates.tile([P, NT, KP, NF], F8)

            # DMA schedule.  Scalar (ACT) queue issues the biases + x halves
            # ahead of its table warmups; the sync ring carries the r-gate
            # critical chain (Wr + h8-n0) so the PE can start ASAP.
            nc.scalar.dma_start(br_sb[:], brd)
            nc.scalar.dma_start(bz_sb[:], bzd)
            nc.scalar.dma_start(bh_sb[:], bhd)
            nc.scalar.dma_start(x8_sb[:, 0], x8d[:, 0])

            # Sync ring: the r-gate critical chain, then the state bytes.
            nc.sync.dma_start(wr_sb[:, 0], Wrd[:, 0])
            nc.sync.dma_start(h8_sb[:, 0], h8d[:, 0])
            nc.sync.dma_start(wr_sb[:, 1], Wrd[:, 1])
            nc.sync.dma_start(wr_sb[:, 2], Wrd[:, 2])
            nc.sync.dma_start(wr_sb[:, 3], Wrd[:, 3])
            nc.sync.dma_start(h8_sb[:, 1], h8d[:, 1])
            nc.sync.dma_start(hpb_sb[:, 0], hpbd[:, 0])
            nc.sync.dma_start(hpb_sb[:, 1], hpbd[:, 1])

            # SWDGE (slow ~6us spin-up) carries everything needed later.
            nc.gpsimd.dma_start(wr_sb[:, 4:8], Wrd[:, 4:8])
            nc.gpsimd.dma_start(wz_sb[:], Wzd)
            if wlo_z:
                nc.gpsimd.dma_start(wzl_sb[:], Wzl)
            nc.gpsimd.dma_start(wh_sb[:], Whd)
            if wlo_h:
                nc.gpsimd.dma_start(whl_sb[:], Whl)

            # Pre-warm both ACT tables during the DMA fill so no real
            # activation pays the table-load latency; x8-n1 is only needed
            # ~10us in, so its issue rides behind the warmups.
            warm = acts.tile([P, 1], F32)
            warm2 = acts.tile([P, 1], F32, name="warm2")
            nc.scalar.activation(warm[:], warm[:], AF.Sigmoid)
            nc.scalar.activation(warm2[:], warm2[:], AF.Tanh)
            nc.scalar.dma_start(x8_sb[:, 1], x8d[:, 1])

            def moving(stage, n, kp, base, width):
                """DoubleRow moving operand [128, 2, width] for pair kp."""
                if kp < KP // 2:
                    src = rh_sb if stage == "h" else h8_sb
                    return src[:, n, 2 * kp:2 * kp + 2, base:base + width]
                kx = 2 * (kp - KP // 2)
                return x8_sb[:, n, kx:kx + 2, base:base + width]

            def mm_group(stage, w_sb, wl_sb, mt, n, ps, base, width):
                nmm = KP if wl_sb is None else 2 * KP
                i = 0
                for w in ([w_sb] if wl_sb is None else [w_sb, wl_sb]):
                    for kp in range(KP):
                        nc.tensor.matmul(
                            ps, w[:, mt, 2 * kp:2 * kp + 2, :],
                            moving(stage, n, kp, base, width),
                            start=(i == 0), stop=(i == nmm - 1), perf_mode=DR)
                        i += 1

            def finish(stage, mt, n, ps, base, width):
                ns = slice(base, base + width)
                if stage == "r":
                    nc.scalar.activation(
                        r_sb[:, n, mt, ns], ps, AF.Sigmoid,
                        bias=br_sb[:, mt:mt + 1], scale=1.0 / WSCALE)
                    nc.vector.tensor_mul(
                        rh_sb[:, n, mt, ns], r_sb[:, n, mt, ns],
                        hpb_sb[:, n, mt, ns])
                elif stage == "z":
                    nc.scalar.activation(
                        z_sb[:, n, mt, ns], ps, AF.Sigmoid,
                        bias=bz_sb[:, mt:mt + 1], scale=1.0 / WSCALE)
                    t1 = opool.tile([P, width], BF16, tag="t1")
                    nc.vector.tensor_mul(
                        t1, z_sb[:, n, mt, ns], hpb_sb[:, n, mt, ns])
                    nc.vector.tensor_sub(
                        c_sb[:, n, mt, ns], hpb_sb[:, n, mt, ns], t1)
                else:
                    ht = opool.tile([P, width], BF16, tag="ht")
                    nc.scalar.activation(
                        ht, ps, AF.Tanh,
                        bias=bh_sb[:, mt:mt + 1], scale=1.0 / WSCALE)
                    t2 = opool.tile([P, width], BF16, tag="t2")
                    nc.vector.tensor_mul(t2, z_sb[:, n, mt, ns], ht)
                    ho = opool.tile([P, width], BF16, tag="ho")
                    nc.vector.tensor_add(ho, c_sb[:, n, mt, ns], t2)
                    nc.sync.dma_start(
                        out[mt * P:(mt + 1) * P,
                            n * NF + base:n * NF + base + width], ho)

            def gate(stage, w_sb, wl_sb):
                if stage == "r":
                    # Ramp: k-outer across (mt0, mt1) at n=0 so the PE
                    # starts on Wr0/Wr1 + h8-n0 only (~1MB landed).
                    NG = 2
                    pss = [ppool.tile([P, NF], F32, tag="ps",
                                      name=f"psri{g}") for g in range(NG)]
                    for kp in range(KP):
                        for g in range(NG):
                            nc.tensor.matmul(
                                pss[g], w_sb[:, g, 2 * kp:2 * kp + 2, :],
                                moving(stage, 0, kp, 0, NF),
                                start=(kp == 0), stop=(kp == KP - 1),
                                perf_mode=DR)
                    for g in range(NG):
                        finish(stage, g, 0, pss[g], 0, NF)
                    plan = ([(mt, 0) for mt in range(NG, MT)]
                            + [(mt, 1) for mt in range(MT)])
                else:
                    plan = ([(mt, 0) for mt in range(MT)]
                            + [(mt, 1) for mt in range(MT)])
                for mt, n in plan:
                    # Split the very last group so its ACT+DVE+store chain
                    # pipelines instead of sitting exposed after the PE ends.
                    last = stage == "h" and mt == MT - 1 and n == NT - 1
                    nsub = 2 if last else 1
                    width = NF // nsub
                    for si in range(nsub):
                        base = si * width
                        ps = ppool.tile([P, width], F32, tag="ps",
                                        name=f"ps{stage}{mt}_{n}_{si}")
                        mm_group(stage, w_sb, wl_sb, mt, n, ps, base, width)
                        finish(stage, mt, n, ps, base, width)

            gate("r", wr_sb, None)
            gate("z", wz_sb, wzl_sb)
            gate("h", wh_sb, whl_sb)

    nc.compile()
    return nc


def _prep_inputs(x, h_prev, W_z, b_z, W_r, b_r, W_h, b_h, variant="fp8"):
    """Host-side relayout: partition-major, fp8/bf16 quantization."""
    import ml_dtypes
    E4 = ml_dtypes.float8_e4m3
    BF = ml_dtypes.bfloat16
    wlo_h = variant in ("fp8wh", "fp8whz")
    wlo_z = variant == "fp8whz"

    def prep_w(W):
        # [p, mt, k, m] <- W[mt*128+m, k*128+p], f32, prescaled
        W4 = W.reshape(MT, P, KT, P)          # [mt, m, k, p]
        Wt = np.ascontiguousarray(W4.transpose(3, 0, 2, 1)) * WSCALE
        Whi = Wt.astype(E4)
        lo = (Wt - Whi.astype(np.float32)).astype(E4)
        return Whi, lo

    def prep_b(b):
        return np.ascontiguousarray(b.reshape(MT, P).T)

    Wr8, Wrl = prep_w(W_r)
    Wz8, Wzl = prep_w(W_z)
    Wh8, Whl = prep_w(W_h)
    shared = {
        "Wr": Wr8, "Wz": Wz8, "Wh": Wh8,
        "br": prep_b(b_r), "bz": prep_b(b_z), "bh": prep_b(b_h),
    }
    if wlo_h:
        shared["Whl"] = Whl
    if wlo_z:
        shared["Wzl"] = Wzl

    def prep_act(a, dt):
        # [p, n, ko, j] <- a[n*NF+j, ko*128+p]  (a is the per-core slice)
        a4 = a.reshape(NT, NF, KP, P)          # [n, j, ko, p]
        return np.ascontiguousarray(a4.transpose(3, 0, 2, 1)).astype(dt)

    in_maps = []
    for c in range(NCORES):
        bs = slice(c * BS, (c + 1) * BS)
        m = dict(shared)
        m["x8"] = prep_act(x[bs], E4)
        m["h8"] = prep_act(h_prev[bs], E4)
        m["hpb"] = prep_act(h_prev[bs], BF)
        in_maps.append(m)
    return in_maps


def run(inputs, mm_dtype="fp8", trace=False, **run_kwargs):
    """Compile + run on 8 cores. Returns (output [B,H] f32, results)."""
    variant = mm_dtype if mm_dtype in ("fp8", "fp8wh", "fp8whz") else "fp8"
    nc = build_kernel(variant)
    in_maps = _prep_inputs(**inputs, variant=variant)
    res = bass_utils.run_bass_kernel_spmd(
        nc, in_maps, core_ids=list(range(NCORES)), trace=trace, **run_kwargs)
    outT = np.concatenate(
        [res.results[c]["out"] for c in range(NCORES)], axis=1)  # [H, B] bf16
    return np.ascontiguousarray(outT.T).astype(np.float32), res


def kernel(**inputs) -> np.ndarray:
    import time as _time
    try:
        out, _ = run(inputs)
    except Exception:
        # The axon-tunneled device occasionally reports a transient
        # "unrecoverable" state right after a crashed session; a fresh
        # attempt after a short pause recovers.
        _time.sleep(15)
        out, _ = run(inputs)
    return out


# revision 6
# speedup vs baseline: 1.7017x; 1.0751x over previous
"""GRU cell kernel for Trainium2, data-parallel across 8 NeuronCores.

Reference computation (per batch row):
    concat = [h_prev, x]                       # [B, 2048]
    z = sigmoid(concat @ W_z.T + b_z)          # [B, 1024]
    r = sigmoid(concat @ W_r.T + b_r)
    h_tilde = tanh([r*h_prev, x] @ W_h.T + b_h)
    h = (1-z)*h_prev + z*h_tilde

Sharding: batch dim (8192) split 1024/core; weights replicated.

Matmuls run in fp8 e4m3 with MatmulPerfMode.DoubleRow: each instruction
consumes TWO 128-deep contraction chunks ([128, 2, free] operands), at
0.5 PE cycles per moving column -> 4x bf16 MAC throughput. Weights are
prescaled by 256 on the host so they sit in e4m3's normal range
(|W| ~ 0.022 would otherwise be denormal); the 1/256 descale folds into
the ACT activation's `scale` operand.  Elementwise runs in bf16 (2x DVE
rate), h_prev is kept as a bf16 copy for the state-carry path, and
c = (1-z)*h_prev is precomputed during the z-stage so the h-stage tail
only needs two DVE ops per tile.  Expected rel err ~1.7e-2 (fp8 operand
quantization, dominated by the h/z gates); `fp8wh`/`fp8whz` variants add
a second weight-residual matmul pass on W_h (and W_z) to cut it to
~1.5e-2/~1.3e-2 at +13.7us PE each.

All DRAM layouts are partition-major so every DMA is contiguous per
partition (>=2KB descriptors): the baseline trace showed the 17us ramp
was descriptor/latency-bound, not bandwidth-bound.
"""

import numpy as np

import concourse.bacc as bacc
import concourse.bass as bass
import concourse.mybir as mybir
import concourse.tile as tile
from concourse import bass_utils

P = 128
B = 8192
I = 1024
H = 1024
K = I + H            # 2048 contraction
NCORES = 8
BS = B // NCORES     # 1024 batch rows per core
MT = H // P          # 8 m-tiles (hidden units)
KT = K // P          # 16 k-chunks
KP = KT // 2         # 8 DoubleRow pairs (0-3 h-part, 4-7 x-part)
NF = 512             # matmul moving free dim (one PSUM bank of fp32)
NT = BS // NF        # 2 n-tiles per core
WSCALE = 256.0       # host-side weight prescale for e4m3 range

F32 = mybir.dt.float32
BF16 = mybir.dt.bfloat16
F8 = mybir.dt.float8e4

AF = mybir.ActivationFunctionType
DR = mybir.MatmulPerfMode.DoubleRow


def build_kernel(variant: str = "fp8"):
    """Build the per-core Bass kernel. Returns compiled nc."""
    wlo_h = variant in ("fp8wh", "fp8whz")
    wlo_z = variant == "fp8whz"
    nc = bacc.Bacc("TRN2", target_bir_lowering=False, debug=False)

    # DRAM I/O (per-core shapes), all partition-major.
    x8d = nc.dram_tensor("x8", [P, NT, KP, NF], F8, kind="ExternalInput").ap()
    h8d = nc.dram_tensor("h8", [P, NT, KP, NF], F8, kind="ExternalInput").ap()
    hpbd = nc.dram_tensor("hpb", [P, NT, MT, NF], BF16,
                          kind="ExternalInput").ap()
    Wrd = nc.dram_tensor("Wr", [P, MT, KT, P], F8, kind="ExternalInput").ap()
    Wzd = nc.dram_tensor("Wz", [P, MT, KT, P], F8, kind="ExternalInput").ap()
    Whd = nc.dram_tensor("Wh", [P, MT, KT, P], F8, kind="ExternalInput").ap()
    Whl = (nc.dram_tensor("Whl", [P, MT, KT, P], F8, kind="ExternalInput").ap()
           if wlo_h else None)
    Wzl = (nc.dram_tensor("Wzl", [P, MT, KT, P], F8, kind="ExternalInput").ap()
           if wlo_z else None)
    brd = nc.dram_tensor("br", [P, MT], F32, kind="ExternalInput").ap()
    bzd = nc.dram_tensor("bz", [P, MT], F32, kind="ExternalInput").ap()
    bhd = nc.dram_tensor("bh", [P, MT], F32, kind="ExternalInput").ap()
    out = nc.dram_tensor("out", [H, BS], BF16, kind="ExternalOutput").ap()

    with tile.TileContext(nc) as tc:
        with (
            tc.tile_pool(name="acts", bufs=1) as acts,
            tc.tile_pool(name="gates", bufs=1) as gates,
            tc.tile_pool(name="opool", bufs=8) as opool,
            tc.tile_pool(name="ppool", bufs=8, space="PSUM") as ppool,
        ):
            # Persistent SBUF state
            x8_sb = acts.tile([P, NT, KP, NF], F8)
            h8_sb = acts.tile([P, NT, KP, NF], F8)
            hpb_sb = acts.tile([P, NT, MT, NF], BF16)
            wr_sb = acts.tile([P, MT, KT, P], F8)
            wz_sb = acts.tile([P, MT, KT, P], F8)
            wh_sb = acts.tile([P, MT, KT, P], F8)
            whl_sb = acts.tile([P, MT, KT, P], F8, name="whl") if wlo_h else None
            wzl_sb = acts.tile([P, MT, KT, P], F8, name="wzl") if wlo_z else None
            br_sb = acts.tile([P, MT], F32)
            bz_sb = acts.tile([P, MT], F32)
            bh_sb = acts.tile([P, MT], F32)
            r_sb = gates.tile([P, NT, MT, NF], BF16)
            z_sb = gates.tile([P, NT, MT, NF], BF16)
            c_sb = gates.tile([P, NT, MT, NF], BF16)
            rh_sb = gates.tile([P, NT, KP, NF], F8)

            # DMA schedule.  A ring drains FIFO, so the sync ring is a strict
            # priority queue: every transfer the PE consumes, in consumption
            # order.  Late/slack loads (Wz, Wh) ride the gpsimd SWDGE ring so
            # they never steal pool bandwidth from the critical chain.
            nc.scalar.dma_start(br_sb[:], brd)
            nc.scalar.dma_start(bz_sb[:], bzd)
            nc.scalar.dma_start(bh_sb[:], bhd)

            nc.sync.dma_start(wr_sb[:, 0], Wrd[:, 0])
            nc.sync.dma_start(h8_sb[:, 0], h8d[:, 0])
            nc.sync.dma_start(wr_sb[:, 1], Wrd[:, 1])
            nc.sync.dma_start(x8_sb[:, 0], x8d[:, 0])
            nc.sync.dma_start(wr_sb[:, 2], Wrd[:, 2])
            nc.sync.dma_start(wr_sb[:, 3], Wrd[:, 3])
            nc.sync.dma_start(wr_sb[:, 4:8], Wrd[:, 4:8])
            nc.sync.dma_start(h8_sb[:, 1], h8d[:, 1])
            nc.sync.dma_start(x8_sb[:, 1], x8d[:, 1])
            nc.sync.dma_start(hpb_sb[:, 0], hpbd[:, 0])
            nc.sync.dma_start(hpb_sb[:, 1], hpbd[:, 1])

            nc.gpsimd.dma_start(wz_sb[:, 0:4], Wzd[:, 0:4])
            nc.gpsimd.dma_start(wz_sb[:, 4:8], Wzd[:, 4:8])
            if wlo_z:
                nc.gpsimd.dma_start(wzl_sb[:], Wzl)
            nc.gpsimd.dma_start(wh_sb[:, 0:4], Whd[:, 0:4])
            nc.gpsimd.dma_start(wh_sb[:, 4:8], Whd[:, 4:8])
            if wlo_h:
                nc.gpsimd.dma_start(whl_sb[:], Whl)

            # Pre-warm both ACT tables during the DMA fill so no real
            # activation pays the table-load latency.
            warm = acts.tile([P, 1], F32)
            warm2 = acts.tile([P, 1], F32, name="warm2")
            nc.scalar.activation(warm[:], warm[:], AF.Sigmoid)
            nc.scalar.activation(warm2[:], warm2[:], AF.Tanh)

            # Pre-warm the PE pstate: dummy DoubleRow matmuls on scratch
            # (uninitialized, never DMA'd -> zero dependencies) keep the PE
            # clocked up through the DMA ramp so the real matmuls start at
            # full speed instead of spending their first ~3us at the mid
            # pstate.  Result is discarded.
            scr_w = acts.tile([P, 2, P], F8, name="scr_w")
            scr_m = acts.tile([P, 2, NF], F8, name="scr_m")
            nc.scalar.memzero(scr_w[:])
            nc.scalar.memzero(scr_m[:])
            scr_ps = ppool.tile([P, NF], F32, tag="ps", name="scr_ps")
            for _ in range(24):
                nc.tensor.matmul(scr_ps, scr_w[:], scr_m[:],
                                 start=True, stop=True, perf_mode=DR)

            def moving(stage, n, kp, base, width):
                """DoubleRow moving operand [128, 2, width] for pair kp."""
                if kp < KP // 2:
                    src = rh_sb if stage == "h" else h8_sb
                    return src[:, n, 2 * kp:2 * kp + 2, base:base + width]
                kx = 2 * (kp - KP // 2)
                return x8_sb[:, n, kx:kx + 2, base:base + width]

            def mm_group(stage, w_sb, wl_sb, mt, n, ps, base, width):
                nmm = KP if wl_sb is None else 2 * KP
                i = 0
                for w in ([w_sb] if wl_sb is None else [w_sb, wl_sb]):
                    for kp in range(KP):
                        nc.tensor.matmul(
                            ps, w[:, mt, 2 * kp:2 * kp + 2, :],
                            moving(stage, n, kp, base, width),
                            start=(i == 0), stop=(i == nmm - 1), perf_mode=DR)
                        i += 1

            def finish(stage, mt, n, ps, base, width):
                ns = slice(base, base + width)
                if stage == "r":
                    nc.scalar.activation(
                        r_sb[:, n, mt, ns], ps, AF.Sigmoid,
                        bias=br_sb[:, mt:mt + 1], scale=1.0 / WSCALE)
                    nc.vector.tensor_mul(
                        rh_sb[:, n, mt, ns], r_sb[:, n, mt, ns],
                        hpb_sb[:, n, mt, ns])
                elif stage == "z":
                    nc.scalar.activation(
                        z_sb[:, n, mt, ns], ps, AF.Sigmoid,
                        bias=bz_sb[:, mt:mt + 1], scale=1.0 / WSCALE)
                    t1 = opool.tile([P, width], BF16, tag="t1")
                    nc.vector.tensor_mul(
                        t1, z_sb[:, n, mt, ns], hpb_sb[:, n, mt, ns])
                    nc.vector.tensor_sub(
                        c_sb[:, n, mt, ns], hpb_sb[:, n, mt, ns], t1)
                else:
                    ht = opool.tile([P, width], BF16, tag="ht")
                    nc.scalar.activation(
                        ht, ps, AF.Tanh,
                        bias=bh_sb[:, mt:mt + 1], scale=1.0 / WSCALE)
                    t2 = opool.tile([P, width], BF16, tag="t2")
                    nc.vector.tensor_mul(t2, z_sb[:, n, mt, ns], ht)
                    ho = opool.tile([P, width], BF16, tag="ho")
                    nc.vector.tensor_add(ho, c_sb[:, n, mt, ns], t2)
                    nc.sync.dma_start(
                        out[mt * P:(mt + 1) * P,
                            n * NF + base:n * NF + base + width], ho)

            def gate(stage, w_sb, wl_sb):
                if stage == "r":
                    # Ramp: k-outer across (mt0, mt1) at n=0 so the PE
                    # starts on Wr0/Wr1 + h8-n0 only (~1MB landed).
                    NG = 2
                    pss = [ppool.tile([P, NF], F32, tag="ps",
                                      name=f"psri{g}") for g in range(NG)]
                    for kp in range(KP):
                        for g in range(NG):
                            nc.tensor.matmul(
                                pss[g], w_sb[:, g, 2 * kp:2 * kp + 2, :],
                                moving(stage, 0, kp, 0, NF),
                                start=(kp == 0), stop=(kp == KP - 1),
                                perf_mode=DR)
                    for g in range(NG):
                        finish(stage, g, 0, pss[g], 0, NF)
                    plan = ([(mt, 0) for mt in range(NG, MT)]
                            + [(mt, 1) for mt in range(MT)])
                else:
                    plan = ([(mt, 0) for mt in range(MT)]
                            + [(mt, 1) for mt in range(MT)])
                for mt, n in plan:
                    # Split the very last group so its ACT+DVE+store chain
                    # pipelines instead of sitting exposed after the PE ends.
                    last = stage == "h" and mt == MT - 1 and n == NT - 1
                    nsub = 2 if last else 1
                    width = NF // nsub
                    for si in range(nsub):
                        base = si * width
                        ps = ppool.tile([P, width], F32, tag="ps",
                                        name=f"ps{stage}{mt}_{n}_{si}")
                        mm_group(stage, w_sb, wl_sb, mt, n, ps, base, width)
                        finish(stage, mt, n, ps, base, width)

            gate("r", wr_sb, None)
            gate("z", wz_sb, wzl_sb)
            gate("h", wh_sb, whl_sb)

    nc.compile()
    return nc


def _prep_inputs(x, h_prev, W_z, b_z, W_r, b_r, W_h, b_h, variant="fp8"):
    """Host-side relayout: partition-major, fp8/bf16 quantization."""
    import ml_dtypes
    E4 = ml_dtypes.float8_e4m3
    BF = ml_dtypes.bfloat16
    wlo_h = variant in ("fp8wh", "fp8whz")
    wlo_z = variant == "fp8whz"

    def prep_w(W):
        # [p, mt, k, m] <- W[mt*128+m, k*128+p], f32, prescaled
        W4 = W.reshape(MT, P, KT, P)          # [mt, m, k, p]
        Wt = np.ascontiguousarray(W4.transpose(3, 0, 2, 1)) * WSCALE
        Whi = Wt.astype(E4)
        lo = (Wt - Whi.astype(np.float32)).astype(E4)
        return Whi, lo

    def prep_b(b):
        return np.ascontiguousarray(b.reshape(MT, P).T)

    Wr8, Wrl = prep_w(W_r)
    Wz8, Wzl = prep_w(W_z)
    Wh8, Whl = prep_w(W_h)
    shared = {
        "Wr": Wr8, "Wz": Wz8, "Wh": Wh8,
        "br": prep_b(b_r), "bz": prep_b(b_z), "bh": prep_b(b_h),
    }
    if wlo_h:
        shared["Whl"] = Whl
    if wlo_z:
        shared["Wzl"] = Wzl

    def prep_act(a, dt):
        # [p, n, ko, j] <- a[n*NF+j, ko*128+p]  (a is the per-core slice)
        a4 = a.reshape(NT, NF, KP, P)          # [n, j, ko, p]
        return np.ascontiguousarray(a4.transpose(3, 0, 2, 1)).astype(dt)

    in_maps = []
    for c in range(NCORES):
        bs = slice(c * BS, (c + 1) * BS)
        m = dict(shared)
        m["x8"] = prep_act(x[bs], E4)
        m["h8"] = prep_act(h_prev[bs], E4)
        m["hpb"] = prep_act(h_prev[bs], BF)
        in_maps.append(m)
    return in_maps


def run(inputs, mm_dtype="fp8", trace=False, **run_kwargs):
    """Compile + run on 8 cores. Returns (output [B,H] f32, results)."""
    variant = mm_dtype if mm_dtype in ("fp8", "fp8wh", "fp8whz") else "fp8"
    nc = build_kernel(variant)
    in_maps = _prep_inputs(**inputs, variant=variant)
    res = bass_utils.run_bass_kernel_spmd(
        nc, in_maps, core_ids=list(range(NCORES)), trace=trace, **run_kwargs)
    outT = np.concatenate(
        [res.results[c]["out"] for c in range(NCORES)], axis=1)  # [H, B] bf16
    return np.ascontiguousarray(outT.T).astype(np.float32), res


def kernel(**inputs) -> np.ndarray:
    import time as _time
    try:
        out, _ = run(inputs)
    except Exception:
        # The axon-tunneled device occasionally reports a transient
        # "unrecoverable" state right after a crashed session; a fresh
        # attempt after a short pause recovers.
        _time.sleep(15)
        out, _ = run(inputs)
    return out


# revision 8
# speedup vs baseline: 1.8713x; 1.0997x over previous
"""GRU cell kernel for Trainium2, data-parallel across 8 NeuronCores.

Reference computation (per batch row):
    concat = [h_prev, x]                       # [B, 2048]
    z = sigmoid(concat @ W_z.T + b_z)          # [B, 1024]
    r = sigmoid(concat @ W_r.T + b_r)
    h_tilde = tanh([r*h_prev, x] @ W_h.T + b_h)
    h = (1-z)*h_prev + z*h_tilde

Sharding: batch dim (8192) split 1024/core; weights replicated.

Matmuls run in fp8 e4m3 with MatmulPerfMode.DoubleRow: each instruction
consumes TWO 128-deep contraction chunks ([128, 2, free] operands), at
0.5 PE cycles per moving column -> 4x bf16 MAC throughput. Weights are
prescaled by 256 on the host so they sit in e4m3's normal range
(|W| ~ 0.022 would otherwise be denormal); the 1/256 descale folds into
the ACT activation's `scale` operand.  Elementwise runs in bf16 (2x DVE
rate), h_prev is kept as a bf16 copy for the state-carry path, and
c = (1-z)*h_prev is precomputed during the z-stage so the h-stage tail
only needs two DVE ops per tile.  Expected rel err ~1.7e-2 (fp8 operand
quantization, dominated by the h/z gates); `fp8wh`/`fp8whz` variants add
a second weight-residual matmul pass on W_h (and W_z) to cut it to
~1.5e-2/~1.3e-2 at +13.7us PE each.

All DRAM layouts are partition-major so every DMA is contiguous per
partition (>=2KB descriptors): the baseline trace showed the 17us ramp
was descriptor/latency-bound, not bandwidth-bound.
"""

import numpy as np

import concourse.bacc as bacc
import concourse.bass as bass
import concourse.mybir as mybir
import concourse.tile as tile
from concourse import bass_utils

P = 128
B = 8192
I = 1024
H = 1024
K = I + H            # 2048 contraction
NCORES = 8
BS = B // NCORES     # 1024 batch rows per core
MT = H // P          # 8 m-tiles (hidden units)
KT = K // P          # 16 k-chunks
KP = KT // 2         # 8 DoubleRow pairs (0-3 h-part, 4-7 x-part)
NF = 512             # matmul moving free dim (one PSUM bank of fp32)
NT = BS // NF        # 2 n-tiles per core
WSCALE = 256.0       # host-side weight prescale for e4m3 range

F32 = mybir.dt.float32
BF16 = mybir.dt.bfloat16
F8 = mybir.dt.float8e4

AF = mybir.ActivationFunctionType
DR = mybir.MatmulPerfMode.DoubleRow


def build_kernel(variant: str = "fp8"):
    """Build the per-core Bass kernel. Returns compiled nc."""
    wlo_h = variant in ("fp8wh", "fp8whz")
    wlo_z = variant == "fp8whz"
    nc = bacc.Bacc("TRN2", target_bir_lowering=False, debug=False)

    # DRAM I/O (per-core shapes), all partition-major.
    x8d = nc.dram_tensor("x8", [P, NT, KP, NF], F8, kind="ExternalInput").ap()
    h8d = nc.dram_tensor("h8", [P, NT, KP, NF], F8, kind="ExternalInput").ap()
    hpbd = nc.dram_tensor("hpb", [P, NT, MT, NF], BF16,
                          kind="ExternalInput").ap()
    Wrd = nc.dram_tensor("Wr", [P, MT, KT, P], F8, kind="ExternalInput").ap()
    Wzd = nc.dram_tensor("Wz", [P, MT, KT, P], F8, kind="ExternalInput").ap()
    Whd = nc.dram_tensor("Wh", [P, MT, KT, P], F8, kind="ExternalInput").ap()
    Whl = (nc.dram_tensor("Whl", [P, MT, KT, P], F8, kind="ExternalInput").ap()
           if wlo_h else None)
    Wzl = (nc.dram_tensor("Wzl", [P, MT, KT, P], F8, kind="ExternalInput").ap()
           if wlo_z else None)
    brd = nc.dram_tensor("br", [P, MT], F32, kind="ExternalInput").ap()
    bzd = nc.dram_tensor("bz", [P, MT], F32, kind="ExternalInput").ap()
    bhd = nc.dram_tensor("bh", [P, MT], F32, kind="ExternalInput").ap()
    out = nc.dram_tensor("out", [H, BS], BF16, kind="ExternalOutput").ap()

    with tile.TileContext(nc) as tc:
        with (
            tc.tile_pool(name="acts", bufs=1) as acts,
            tc.tile_pool(name="gates", bufs=1) as gates,
            tc.tile_pool(name="opool", bufs=8) as opool,
            tc.tile_pool(name="ppool", bufs=8, space="PSUM") as ppool,
        ):
            # Persistent SBUF state
            x8_sb = acts.tile([P, NT, KP, NF], F8)
            h8_sb = acts.tile([P, NT, KP, NF], F8)
            hpb_sb = acts.tile([P, NT, MT, NF], BF16)
            wr_sb = acts.tile([P, MT, KT, P], F8)
            wz_sb = acts.tile([P, MT, KT, P], F8)
            wh_sb = acts.tile([P, MT, KT, P], F8)
            whl_sb = acts.tile([P, MT, KT, P], F8, name="whl") if wlo_h else None
            wzl_sb = acts.tile([P, MT, KT, P], F8, name="wzl") if wlo_z else None
            br_sb = acts.tile([P, MT], F32)
            bz_sb = acts.tile([P, MT], F32)
            bh_sb = acts.tile([P, MT], F32)
            r_sb = gates.tile([P, NT, MT, NF], BF16)
            z_sb = gates.tile([P, NT, MT, NF], BF16)
            c_sb = gates.tile([P, NT, MT, NF], BF16)
            rh_sb = gates.tile([P, NT, KP, NF], F8)

            # DMA schedule.  A ring drains FIFO, so the sync ring is a strict
            # priority queue: every transfer the PE consumes, in consumption
            # order.  Late/slack loads (Wz, Wh) ride the gpsimd SWDGE ring so
            # they never steal pool bandwidth from the critical chain.
            nc.scalar.dma_start(br_sb[:], brd)
            nc.scalar.dma_start(bz_sb[:], bzd)
            nc.scalar.dma_start(bh_sb[:], bhd)

            nc.sync.dma_start(wr_sb[:, 0], Wrd[:, 0])
            nc.sync.dma_start(h8_sb[:, 0], h8d[:, 0])
            nc.sync.dma_start(wr_sb[:, 1], Wrd[:, 1])
            nc.sync.dma_start(x8_sb[:, 0], x8d[:, 0])
            nc.sync.dma_start(wr_sb[:, 2], Wrd[:, 2])
            nc.sync.dma_start(wr_sb[:, 3], Wrd[:, 3])
            nc.sync.dma_start(wr_sb[:, 4:8], Wrd[:, 4:8])
            nc.sync.dma_start(h8_sb[:, 1], h8d[:, 1])
            nc.sync.dma_start(x8_sb[:, 1], x8d[:, 1])
            nc.sync.dma_start(wz_sb[:, 0:4], Wzd[:, 0:4])
            nc.sync.dma_start(wz_sb[:, 4:8], Wzd[:, 4:8])
            if wlo_z:
                nc.sync.dma_start(wzl_sb[:], Wzl)
            nc.sync.dma_start(hpb_sb[:, 0], hpbd[:, 0])
            nc.sync.dma_start(wh_sb[:, 0:4], Whd[:, 0:4])
            nc.sync.dma_start(wh_sb[:, 4:8], Whd[:, 4:8])
            if wlo_h:
                nc.sync.dma_start(whl_sb[:], Whl)
            nc.sync.dma_start(hpb_sb[:, 1], hpbd[:, 1])

            # Pre-warm both ACT tables during the DMA fill so no real
            # activation pays the table-load latency.
            warm = acts.tile([P, 1], F32)
            warm2 = acts.tile([P, 1], F32, name="warm2")
            nc.scalar.activation(warm[:], warm[:], AF.Sigmoid)
            nc.scalar.activation(warm2[:], warm2[:], AF.Tanh)

            # Pre-warm the PE pstate: dummy DoubleRow matmuls on scratch
            # (uninitialized, never DMA'd -> zero dependencies) keep the PE
            # clocked up through the DMA ramp so the real matmuls start at
            # full speed instead of spending their first ~3us at the mid
            # pstate.  Result is discarded.
            scr_w = acts.tile([P, 2, P], F8, name="scr_w")
            scr_m = acts.tile([P, 2, NF], F8, name="scr_m")
            nc.scalar.memzero(scr_w[:])
            nc.scalar.memzero(scr_m[:])
            scr_ps = ppool.tile([P, NF], F32, tag="ps", name="scr_ps")
            for _ in range(8):
                nc.tensor.matmul(scr_ps, scr_w[:], scr_m[:],
                                 start=True, stop=True, perf_mode=DR)

            def moving(stage, n, kp, base, width):
                """DoubleRow moving operand [128, 2, width] for pair kp."""
                if kp < KP // 2:
                    src = rh_sb if stage == "h" else h8_sb
                    return src[:, n, 2 * kp:2 * kp + 2, base:base + width]
                kx = 2 * (kp - KP // 2)
                return x8_sb[:, n, kx:kx + 2, base:base + width]

            def mm_group(stage, w_sb, wl_sb, mt, n, ps, base, width):
                nmm = KP if wl_sb is None else 2 * KP
                i = 0
                for w in ([w_sb] if wl_sb is None else [w_sb, wl_sb]):
                    for kp in range(KP):
                        nc.tensor.matmul(
                            ps, w[:, mt, 2 * kp:2 * kp + 2, :],
                            moving(stage, n, kp, base, width),
                            start=(i == 0), stop=(i == nmm - 1), perf_mode=DR)
                        i += 1

            def finish(stage, mt, n, ps, base, width):
                ns = slice(base, base + width)
                if stage == "r":
                    nc.scalar.activation(
                        r_sb[:, n, mt, ns], ps, AF.Sigmoid,
                        bias=br_sb[:, mt:mt + 1], scale=1.0 / WSCALE)
                    nc.vector.tensor_mul(
                        rh_sb[:, n, mt, ns], r_sb[:, n, mt, ns],
                        hpb_sb[:, n, mt, ns])
                elif stage == "z":
                    nc.scalar.activation(
                        z_sb[:, n, mt, ns], ps, AF.Sigmoid,
                        bias=bz_sb[:, mt:mt + 1], scale=1.0 / WSCALE)
                    t1 = opool.tile([P, width], BF16, tag="t1")
                    nc.vector.tensor_mul(
                        t1, z_sb[:, n, mt, ns], hpb_sb[:, n, mt, ns])
                    nc.vector.tensor_sub(
                        c_sb[:, n, mt, ns], hpb_sb[:, n, mt, ns], t1)
                else:
                    ht = opool.tile([P, width], BF16, tag="ht")
                    nc.scalar.activation(
                        ht, ps, AF.Tanh,
                        bias=bh_sb[:, mt:mt + 1], scale=1.0 / WSCALE)
                    t2 = opool.tile([P, width], BF16, tag="t2")
                    nc.vector.tensor_mul(t2, z_sb[:, n, mt, ns], ht)
                    ho = opool.tile([P, width], BF16, tag="ho")
                    nc.vector.tensor_add(ho, c_sb[:, n, mt, ns], t2)
                    nc.sync.dma_start(
                        out[mt * P:(mt + 1) * P,
                            n * NF + base:n * NF + base + width], ho)

            def gate(stage, w_sb, wl_sb):
                if stage == "r":
                    # Ramp: k-outer across (mt0, mt1) at n=0 so the PE
                    # starts on Wr0/Wr1 + h8-n0 only (~1MB landed).
                    NG = 2
                    pss = [ppool.tile([P, NF], F32, tag="ps",
                                      name=f"psri{g}") for g in range(NG)]
                    for kp in range(KP):
                        for g in range(NG):
                            nc.tensor.matmul(
                                pss[g], w_sb[:, g, 2 * kp:2 * kp + 2, :],
                                moving(stage, 0, kp, 0, NF),
                                start=(kp == 0), stop=(kp == KP - 1),
                                perf_mode=DR)
                    for g in range(NG):
                        finish(stage, g, 0, pss[g], 0, NF)
                    plan = ([(mt, 0) for mt in range(NG, MT)]
                            + [(mt, 1) for mt in range(MT)])
                else:
                    plan = ([(mt, 0) for mt in range(MT)]
                            + [(mt, 1) for mt in range(MT)])
                for mt, n in plan:
                    # Split the very last group so its ACT+DVE+store chain
                    # pipelines instead of sitting exposed after the PE ends.
                    last = stage == "h" and mt == MT - 1 and n == NT - 1
                    nsub = 2 if last else 1
                    width = NF // nsub
                    for si in range(nsub):
                        base = si * width
                        ps = ppool.tile([P, width], F32, tag="ps",
                                        name=f"ps{stage}{mt}_{n}_{si}")
                        mm_group(stage, w_sb, wl_sb, mt, n, ps, base, width)
                        finish(stage, mt, n, ps, base, width)

            gate("r", wr_sb, None)
            gate("z", wz_sb, wzl_sb)
            gate("h", wh_sb, whl_sb)

    nc.compile()
    return nc


def _prep_inputs(x, h_prev, W_z, b_z, W_r, b_r, W_h, b_h, variant="fp8"):
    """Host-side relayout: partition-major, fp8/bf16 quantization."""
    import ml_dtypes
    E4 = ml_dtypes.float8_e4m3
    BF = ml_dtypes.bfloat16
    wlo_h = variant in ("fp8wh", "fp8whz")
    wlo_z = variant == "fp8whz"

    def prep_w(W):
        # [p, mt, k, m] <- W[mt*128+m, k*128+p], f32, prescaled
        W4 = W.reshape(MT, P, KT, P)          # [mt, m, k, p]
        Wt = np.ascontiguousarray(W4.transpose(3, 0, 2, 1)) * WSCALE
        Whi = Wt.astype(E4)
        lo = (Wt - Whi.astype(np.float32)).astype(E4)
        return Whi, lo

    def prep_b(b):
        return np.ascontiguousarray(b.reshape(MT, P).T)

    Wr8, Wrl = prep_w(W_r)
    Wz8, Wzl = prep_w(W_z)
    Wh8, Whl = prep_w(W_h)
    shared = {
        "Wr": Wr8, "Wz": Wz8, "Wh": Wh8,
        "br": prep_b(b_r), "bz": prep_b(b_z), "bh": prep_b(b_h),
    }
    if wlo_h:
        shared["Whl"] = Whl
    if wlo_z:
        shared["Wzl"] = Wzl

    def prep_act(a, dt):
        # [p, n, ko, j] <- a[n*NF+j, ko*128+p]  (a is the per-core slice)
        a4 = a.reshape(NT, NF, KP, P)          # [n, j, ko, p]
        return np.ascontiguousarray(a4.transpose(3, 0, 2, 1)).astype(dt)

    in_maps = []
    for c in range(NCORES):
        bs = slice(c * BS, (c + 1) * BS)
        m = dict(shared)
        m["x8"] = prep_act(x[bs], E4)
        m["h8"] = prep_act(h_prev[bs], E4)
        m["hpb"] = prep_act(h_prev[bs], BF)
        in_maps.append(m)
    return in_maps


def run(inputs, mm_dtype="fp8", trace=False, **run_kwargs):
    """Compile + run on 8 cores. Returns (output [B,H] f32, results)."""
    variant = mm_dtype if mm_dtype in ("fp8", "fp8wh", "fp8whz") else "fp8"
    nc = build_kernel(variant)
    in_maps = _prep_inputs(**inputs, variant=variant)
    res = bass_utils.run_bass_kernel_spmd(
        nc, in_maps, core_ids=list(range(NCORES)), trace=trace, **run_kwargs)
    outT = np.concatenate(
        [res.results[c]["out"] for c in range(NCORES)], axis=1)  # [H, B] bf16
    return np.ascontiguousarray(outT.T).astype(np.float32), res


def kernel(**inputs) -> np.ndarray:
    import time as _time
    try:
        out, _ = run(inputs)
    except Exception:
        # The axon-tunneled device occasionally reports a transient
        # "unrecoverable" state right after a crashed session; a fresh
        # attempt after a short pause recovers.
        _time.sleep(15)
        out, _ = run(inputs)
    return out


# revision 10
# speedup vs baseline: 1.8908x; 1.0104x over previous
"""GRU cell kernel for Trainium2, data-parallel across 8 NeuronCores.

Reference computation (per batch row):
    concat = [h_prev, x]                       # [B, 2048]
    z = sigmoid(concat @ W_z.T + b_z)          # [B, 1024]
    r = sigmoid(concat @ W_r.T + b_r)
    h_tilde = tanh([r*h_prev, x] @ W_h.T + b_h)
    h = (1-z)*h_prev + z*h_tilde

Sharding: batch dim (8192) split 1024/core; weights replicated.

Matmuls run in fp8 e4m3 with MatmulPerfMode.DoubleRow: each instruction
consumes TWO 128-deep contraction chunks ([128, 2, free] operands), at
0.5 PE cycles per moving column -> 4x bf16 MAC throughput. Weights are
prescaled by 256 on the host so they sit in e4m3's normal range
(|W| ~ 0.022 would otherwise be denormal); the 1/256 descale folds into
the ACT activation's `scale` operand.  Elementwise runs in bf16 (2x DVE
rate), h_prev is kept as a bf16 copy for the state-carry path, and
c = (1-z)*h_prev is precomputed during the z-stage so the h-stage tail
only needs two DVE ops per tile.  Expected rel err ~1.7e-2 (fp8 operand
quantization, dominated by the h/z gates); `fp8wh`/`fp8whz` variants add
a second weight-residual matmul pass on W_h (and W_z) to cut it to
~1.5e-2/~1.3e-2 at +13.7us PE each.

All DRAM layouts are partition-major so every DMA is contiguous per
partition (>=2KB descriptors): the baseline trace showed the 17us ramp
was descriptor/latency-bound, not bandwidth-bound.
"""

import numpy as np

import concourse.bacc as bacc
import concourse.bass as bass
import concourse.mybir as mybir
import concourse.tile as tile
from concourse import bass_utils

P = 128
B = 8192
I = 1024
H = 1024
K = I + H            # 2048 contraction
NCORES = 8
BS = B // NCORES     # 1024 batch rows per core
MT = H // P          # 8 m-tiles (hidden units)
KT = K // P          # 16 k-chunks
KP = KT // 2         # 8 DoubleRow pairs (0-3 h-part, 4-7 x-part)
NF = 512             # matmul moving free dim (one PSUM bank of fp32)
NT = BS // NF        # 2 n-tiles per core
WSCALE = 256.0       # host-side weight prescale for e4m3 range

F32 = mybir.dt.float32
BF16 = mybir.dt.bfloat16
F8 = mybir.dt.float8e4

AF = mybir.ActivationFunctionType
DR = mybir.MatmulPerfMode.DoubleRow


def build_kernel(variant: str = "fp8"):
    """Build the per-core Bass kernel. Returns compiled nc."""
    wlo_h = variant in ("fp8wh", "fp8whz")
    wlo_z = variant == "fp8whz"
    nc = bacc.Bacc("TRN2", target_bir_lowering=False, debug=False)

    # DRAM I/O (per-core shapes), all partition-major.
    x8d = nc.dram_tensor("x8", [P, NT, KP, NF], F8, kind="ExternalInput").ap()
    h8d = nc.dram_tensor("h8", [P, NT, KP, NF], F8, kind="ExternalInput").ap()
    hpbd = nc.dram_tensor("hpb", [P, NT, MT, NF], BF16,
                          kind="ExternalInput").ap()
    Wrd = nc.dram_tensor("Wr", [P, MT, KT, P], F8, kind="ExternalInput").ap()
    Wzd = nc.dram_tensor("Wz", [P, MT, KT, P], F8, kind="ExternalInput").ap()
    Whd = nc.dram_tensor("Wh", [P, MT, KT, P], F8, kind="ExternalInput").ap()
    Whl = (nc.dram_tensor("Whl", [P, MT, KT, P], F8, kind="ExternalInput").ap()
           if wlo_h else None)
    Wzl = (nc.dram_tensor("Wzl", [P, MT, KT, P], F8, kind="ExternalInput").ap()
           if wlo_z else None)
    brd = nc.dram_tensor("br", [P, MT], F32, kind="ExternalInput").ap()
    bzd = nc.dram_tensor("bz", [P, MT], F32, kind="ExternalInput").ap()
    bhd = nc.dram_tensor("bh", [P, MT], F32, kind="ExternalInput").ap()
    out = nc.dram_tensor("out", [H, BS], BF16, kind="ExternalOutput").ap()

    with tile.TileContext(nc) as tc:
        with (
            tc.tile_pool(name="acts", bufs=1) as acts,
            tc.tile_pool(name="gates", bufs=1) as gates,
            tc.tile_pool(name="opool", bufs=8) as opool,
            tc.tile_pool(name="ppool", bufs=8, space="PSUM") as ppool,
        ):
            # Persistent SBUF state
            x8_sb = acts.tile([P, NT, KP, NF], F8)
            h8_sb = acts.tile([P, NT, KP, NF], F8)
            hpb_sb = acts.tile([P, NT, MT, NF], BF16)
            wr_sb = acts.tile([P, MT, KT, P], F8)
            wz_sb = acts.tile([P, MT, KT, P], F8)
            wh_sb = acts.tile([P, MT, KT, P], F8)
            whl_sb = acts.tile([P, MT, KT, P], F8, name="whl") if wlo_h else None
            wzl_sb = acts.tile([P, MT, KT, P], F8, name="wzl") if wlo_z else None
            br_sb = acts.tile([P, MT], F32)
            bz_sb = acts.tile([P, MT], F32)
            bh_sb = acts.tile([P, MT], F32)
            r_sb = gates.tile([P, NT, MT, NF], BF16)
            z_sb = gates.tile([P, NT, MT, NF], BF16)
            c_sb = gates.tile([P, NT, MT, NF], BF16)
            rh_sb = gates.tile([P, NT, KP, NF], F8)

            # PE pstate pre-warm scratch, zeroed on the (otherwise idle)
            # vector queue so the dummy matmuls can start ~4us in.
            scr_w = acts.tile([P, 2, P], F8, name="scr_w")
            scr_m = acts.tile([P, 2, NF], F8, name="scr_m")
            nc.vector.memset(scr_w[:], 0)
            nc.vector.memset(scr_m[:], 0)

            # DMA schedule.  A ring drains FIFO, so the sync ring is a strict
            # priority queue: every transfer the PE consumes, in consumption
            # order.
            nc.scalar.dma_start(br_sb[:], brd)
            nc.scalar.dma_start(bz_sb[:], bzd)
            nc.scalar.dma_start(bh_sb[:], bhd)

            nc.sync.dma_start(wr_sb[:, 0], Wrd[:, 0])
            nc.sync.dma_start(h8_sb[:, 0, 0:4], h8d[:, 0, 0:4])
            nc.sync.dma_start(wr_sb[:, 1], Wrd[:, 1])
            nc.sync.dma_start(h8_sb[:, 0, 4:8], h8d[:, 0, 4:8])
            nc.sync.dma_start(x8_sb[:, 0], x8d[:, 0])
            nc.sync.dma_start(wr_sb[:, 2], Wrd[:, 2])
            nc.sync.dma_start(wr_sb[:, 3], Wrd[:, 3])
            nc.sync.dma_start(wr_sb[:, 4:8], Wrd[:, 4:8])
            nc.sync.dma_start(h8_sb[:, 1], h8d[:, 1])
            nc.sync.dma_start(x8_sb[:, 1], x8d[:, 1])
            nc.sync.dma_start(wz_sb[:, 0:4], Wzd[:, 0:4])
            nc.sync.dma_start(wz_sb[:, 4:8], Wzd[:, 4:8])
            if wlo_z:
                nc.sync.dma_start(wzl_sb[:], Wzl)
            nc.sync.dma_start(hpb_sb[:, 0], hpbd[:, 0])
            nc.sync.dma_start(wh_sb[:, 0:4], Whd[:, 0:4])
            nc.sync.dma_start(wh_sb[:, 4:8], Whd[:, 4:8])
            if wlo_h:
                nc.sync.dma_start(whl_sb[:], Whl)
            nc.sync.dma_start(hpb_sb[:, 1], hpbd[:, 1])

            # Pre-warm both ACT tables during the DMA fill so no real
            # activation pays the table-load latency.
            warm = acts.tile([P, 1], F32)
            warm2 = acts.tile([P, 1], F32, name="warm2")
            nc.scalar.activation(warm[:], warm[:], AF.Sigmoid)
            nc.scalar.activation(warm2[:], warm2[:], AF.Tanh)

            # Pre-warm the PE pstate: dummy DoubleRow matmuls on the zeroed
            # scratch keep the PE clocked up through the DMA ramp so the real
            # matmuls start at full speed instead of spending their first
            # ~3us at the mid pstate.  Result is discarded.
            scr_ps = ppool.tile([P, NF], F32, tag="ps", name="scr_ps")
            for _ in range(12):
                nc.tensor.matmul(scr_ps, scr_w[:], scr_m[:],
                                 start=True, stop=True, perf_mode=DR)

            def moving(stage, n, kp, base, width):
                """DoubleRow moving operand [128, 2, width] for pair kp."""
                if kp < KP // 2:
                    src = rh_sb if stage == "h" else h8_sb
                    return src[:, n, 2 * kp:2 * kp + 2, base:base + width]
                kx = 2 * (kp - KP // 2)
                return x8_sb[:, n, kx:kx + 2, base:base + width]

            def mm_group(stage, w_sb, wl_sb, mt, n, ps, base, width):
                nmm = KP if wl_sb is None else 2 * KP
                i = 0
                for w in ([w_sb] if wl_sb is None else [w_sb, wl_sb]):
                    for kp in range(KP):
                        nc.tensor.matmul(
                            ps, w[:, mt, 2 * kp:2 * kp + 2, :],
                            moving(stage, n, kp, base, width),
                            start=(i == 0), stop=(i == nmm - 1), perf_mode=DR)
                        i += 1

            def finish(stage, mt, n, ps, base, width):
                ns = slice(base, base + width)
                if stage == "r":
                    nc.scalar.activation(
                        r_sb[:, n, mt, ns], ps, AF.Sigmoid,
                        bias=br_sb[:, mt:mt + 1], scale=1.0 / WSCALE)
                    nc.vector.tensor_mul(
                        rh_sb[:, n, mt, ns], r_sb[:, n, mt, ns],
                        hpb_sb[:, n, mt, ns])
                elif stage == "z":
                    nc.scalar.activation(
                        z_sb[:, n, mt, ns], ps, AF.Sigmoid,
                        bias=bz_sb[:, mt:mt + 1], scale=1.0 / WSCALE)
                    t1 = opool.tile([P, width], BF16, tag="t1")
                    nc.vector.tensor_mul(
                        t1, z_sb[:, n, mt, ns], hpb_sb[:, n, mt, ns])
                    nc.vector.tensor_sub(
                        c_sb[:, n, mt, ns], hpb_sb[:, n, mt, ns], t1)
                else:
                    ht = opool.tile([P, width], BF16, tag="ht")
                    nc.scalar.activation(
                        ht, ps, AF.Tanh,
                        bias=bh_sb[:, mt:mt + 1], scale=1.0 / WSCALE)
                    t2 = opool.tile([P, width], BF16, tag="t2")
                    nc.vector.tensor_mul(t2, z_sb[:, n, mt, ns], ht)
                    ho = opool.tile([P, width], BF16, tag="ho")
                    nc.vector.tensor_add(ho, c_sb[:, n, mt, ns], t2)
                    nc.sync.dma_start(
                        out[mt * P:(mt + 1) * P,
                            n * NF + base:n * NF + base + width], ho)

            def gate(stage, w_sb, wl_sb):
                if stage == "r":
                    # Ramp: k-outer across (mt0, mt1) at n=0 so the PE
                    # starts on Wr0/Wr1 + h8-n0 only (~1MB landed).
                    NG = 2
                    pss = [ppool.tile([P, NF], F32, tag="ps",
                                      name=f"psri{g}") for g in range(NG)]
                    for kp in range(KP):
                        for g in range(NG):
                            nc.tensor.matmul(
                                pss[g], w_sb[:, g, 2 * kp:2 * kp + 2, :],
                                moving(stage, 0, kp, 0, NF),
                                start=(kp == 0), stop=(kp == KP - 1),
                                perf_mode=DR)
                    for g in range(NG):
                        finish(stage, g, 0, pss[g], 0, NF)
                    plan = ([(mt, 0) for mt in range(NG, MT)]
                            + [(mt, 1) for mt in range(MT)])
                else:
                    plan = ([(mt, 0) for mt in range(MT)]
                            + [(mt, 1) for mt in range(MT)])
                for mt, n in plan:
                    # Split the very last group so its ACT+DVE+store chain
                    # pipelines instead of sitting exposed after the PE ends.
                    last = stage == "h" and mt == MT - 1 and n == NT - 1
                    nsub = 2 if last else 1
                    width = NF // nsub
                    for si in range(nsub):
                        base = si * width
                        ps = ppool.tile([P, width], F32, tag="ps",
                                        name=f"ps{stage}{mt}_{n}_{si}")
                        mm_group(stage, w_sb, wl_sb, mt, n, ps, base, width)
                        finish(stage, mt, n, ps, base, width)

            gate("r", wr_sb, None)
            gate("z", wz_sb, wzl_sb)
            gate("h", wh_sb, whl_sb)

    nc.compile()
    return nc


def _prep_inputs(x, h_prev, W_z, b_z, W_r, b_r, W_h, b_h, variant="fp8"):
    """Host-side relayout: partition-major, fp8/bf16 quantization."""
    import ml_dtypes
    E4 = ml_dtypes.float8_e4m3
    BF = ml_dtypes.bfloat16
    wlo_h = variant in ("fp8wh", "fp8whz")
    wlo_z = variant == "fp8whz"

    def prep_w(W):
        # [p, mt, k, m] <- W[mt*128+m, k*128+p], f32, prescaled
        W4 = W.reshape(MT, P, KT, P)          # [mt, m, k, p]
        Wt = np.ascontiguousarray(W4.transpose(3, 0, 2, 1)) * WSCALE
        Whi = Wt.astype(E4)
        lo = (Wt - Whi.astype(np.float32)).astype(E4)
        return Whi, lo

    def prep_b(b):
        return np.ascontiguousarray(b.reshape(MT, P).T)

    Wr8, Wrl = prep_w(W_r)
    Wz8, Wzl = prep_w(W_z)
    Wh8, Whl = prep_w(W_h)
    shared = {
        "Wr": Wr8, "Wz": Wz8, "Wh": Wh8,
        "br": prep_b(b_r), "bz": prep_b(b_z), "bh": prep_b(b_h),
    }
    if wlo_h:
        shared["Whl"] = Whl
    if wlo_z:
        shared["Wzl"] = Wzl

    def prep_act(a, dt):
        # [p, n, ko, j] <- a[n*NF+j, ko*128+p]  (a is the per-core slice)
        a4 = a.reshape(NT, NF, KP, P)          # [n, j, ko, p]
        return np.ascontiguousarray(a4.transpose(3, 0, 2, 1)).astype(dt)

    in_maps = []
    for c in range(NCORES):
        bs = slice(c * BS, (c + 1) * BS)
        m = dict(shared)
        m["x8"] = prep_act(x[bs], E4)
        m["h8"] = prep_act(h_prev[bs], E4)
        m["hpb"] = prep_act(h_prev[bs], BF)
        in_maps.append(m)
    return in_maps


def run(inputs, mm_dtype="fp8", trace=False, **run_kwargs):
    """Compile + run on 8 cores. Returns (output [B,H] f32, results)."""
    variant = mm_dtype if mm_dtype in ("fp8", "fp8wh", "fp8whz") else "fp8"
    nc = build_kernel(variant)
    in_maps = _prep_inputs(**inputs, variant=variant)
    res = bass_utils.run_bass_kernel_spmd(
        nc, in_maps, core_ids=list(range(NCORES)), trace=trace, **run_kwargs)
    outT = np.concatenate(
        [res.results[c]["out"] for c in range(NCORES)], axis=1)  # [H, B] bf16
    return np.ascontiguousarray(outT.T).astype(np.float32), res


def kernel(**inputs) -> np.ndarray:
    import time as _time
    try:
        out, _ = run(inputs)
    except Exception:
        # The axon-tunneled device occasionally reports a transient
        # "unrecoverable" state right after a crashed session; a fresh
        # attempt after a short pause recovers.
        _time.sleep(15)
        out, _ = run(inputs)
    return out
